# revision 5
# baseline (speedup 1.0000x reference)
"""Trainium2 Bass kernel for CustomEmbeddings (embedding lookup + masked MLP).

Computation (reference):
    emb = emb_table[input_ids]                    # [B, S, D]
    mask = input_ids >= 32000
    h = relu(emb @ w1 + b1); mlp = h @ w2 + b2
    out = where(mask, mlp, emb)

Strategy (8 NeuronCores):
  - Data-parallel over tokens: 16384 tokens -> 2048/core; emb table replicated.
    Each core gathers its 2048 rows (indirect DMA) and streams them to its
    output slice. No collectives.
  - The masked-token MLP is tiny (~51 tokens expected). It is weight-sharded
    8-way: core c computes h[:, c*800:(c+1)*800] = relu(emb@w1_c + b1_c) and a
    partial mlp_out = h_c @ w2_c.  The 8 partial sums ([K,3200] each, ~650KB)
    are reduced on the host during unsharding, + b2, and scattered into the
    masked rows of the final output.
"""

import sys

if "/opt/trn_rl_repo" not in sys.path:
    sys.path.insert(0, "/opt/trn_rl_repo")

import numpy as np

from concourse import bacc, bass, mybir
import concourse.tile as tile
from concourse.bass_utils import run_bass_kernel_spmd
from concourse.masks import make_identity

P = 128
VOCAB = 32100
DIM = 3200
HID = 6400
NEW_START = 32000
N_CORES = 8
SHARD_HID = HID // N_CORES          # 800
T_PER_CORE = 2048
N_T_CHUNKS = T_PER_CORE // P        # 16
N_K_TILES = DIM // P                # 25


def cdiv(a, b):
    return (a + b - 1) // b


def build_program(n_mlp_chunks: int) -> bass.Bass:
    f32 = mybir.dt.float32
    i32 = mybir.dt.int32

    # Bacc (not plain Bass): its finalize() runs the wait-legalization passes
    # (move_matmul_waits_to_ldweights / generate_event_semaphores) that split
    # multi-wait instructions the TRN2 ISA encodings cannot carry.
    nc = bacc.Bacc("TRN2")
    ids_t = nc.declare_dram_parameter("ids_t", [P, N_T_CHUNKS], i32, isOutput=False)
    mlp_ids = nc.declare_dram_parameter(
        "mlp_ids", [P, n_mlp_chunks], i32, isOutput=False
    )
    table = nc.declare_dram_parameter("table", [VOCAB, DIM], f32, isOutput=False)
    w1s = nc.declare_dram_parameter("w1s", [DIM, SHARD_HID], f32, isOutput=False)
    b1s = nc.declare_dram_parameter("b1s", [1, SHARD_HID], f32, isOutput=False)
    w2s = nc.declare_dram_parameter("w2s", [SHARD_HID, DIM], f32, isOutput=False)
    out_main = nc.declare_dram_parameter(
        "out_main", [T_PER_CORE, DIM], f32, isOutput=True
    )
    mlp_part = nc.declare_dram_parameter(
        "mlp_part", [n_mlp_chunks * P, DIM], f32, isOutput=True
    )

    n_hb = cdiv(SHARD_HID, P)  # 7 blocks of h columns (6 full + 32)

    with tile.TileContext(nc) as tc:
        with (
            tc.tile_pool(name="const", bufs=1) as consts,
            tc.tile_pool(name="gpool", bufs=4) as gpool,
            tc.tile_pool(name="mpool", bufs=1) as mpool,
            tc.tile_pool(name="wpool", bufs=2) as wpool,
            tc.tile_pool(name="psA", bufs=2, space="PSUM") as psA,
            tc.tile_pool(name="psH", bufs=1, space="PSUM") as psH,
            tc.tile_pool(name="psO", bufs=1, space="PSUM") as psO,
        ):
            ones_row = consts.tile([1, P], f32)
            nc.gpsimd.memset(ones_row[:], 1.0)
            identity = consts.tile([P, P], f32)
            make_identity(nc, identity[:])
            # Priming transpose: the PE transpose lowers to a pure LW
            # instruction that supports only ONE sync wait.  This op makes PE
            # observe the Pool semaphore (identity/ones memsets), so later
            # transposes only wait on their data input.
            prime = psA.tile([P, P], f32, space="PSUM", tag="tp")
            nc.tensor.transpose(out=prime[:], in_=identity[:], identity=identity[:])

            idx_sb = consts.tile([P, N_T_CHUNKS], i32)
            nc.sync.dma_start(out=idx_sb[:], in_=ids_t[:])
            midx_sb = consts.tile([P, n_mlp_chunks], i32)
            nc.sync.dma_start(out=midx_sb[:], in_=mlp_ids[:])
            b1_sb = consts.tile([1, SHARD_HID], f32)
            nc.sync.dma_start(out=b1_sb[:], in_=b1s[:])

            # ---------------- masked-token MLP (small; overlaps with gather) ----
            for j in range(n_mlp_chunks):
                memb = mpool.tile([P, DIM], f32, tag="memb")
                nc.gpsimd.indirect_dma_start(
                    out=memb[:],
                    out_offset=None,
                    in_=table[:],
                    in_offset=bass.IndirectOffsetOnAxis(
                        ap=midx_sb[:, j : j + 1], axis=0
                    ),
                )
                # embT[p, k*P + t] = memb[t, k*P + p]
                embT = mpool.tile([P, DIM], f32, tag="embT")
                for k in range(N_K_TILES):
                    tp = psA.tile([P, P], f32, space="PSUM", tag="tp")
                    nc.tensor.transpose(
                        out=tp[:], in_=memb[:, k * P : (k + 1) * P], identity=identity[:]
                    )
                    nc.vector.tensor_copy(out=embT[:, k * P : (k + 1) * P], in_=tp[:])

                # L1: h = relu(emb @ w1s + b1s), h in [tokens, SHARD_HID]
                hps = psH.tile([P, SHARD_HID], f32, space="PSUM", tag="hps")
                for k in range(N_K_TILES):
                    w1k = wpool.tile([P, SHARD_HID], f32, tag="w1k")
                    nc.sync.dma_start(out=w1k[:], in_=w1s[k * P : (k + 1) * P, :])
                    for n0 in range(0, SHARD_HID, 512):
                        n1 = min(n0 + 512, SHARD_HID)
                        nc.tensor.matmul(
                            hps[:, n0:n1],
                            lhsT=embT[:, k * P : (k + 1) * P],
                            rhs=w1k[:, n0:n1],
                            start=(k == 0),
                            stop=False,
                        )
                # bias add as rank-1 update: ones[tokens] x b1[cols]
                for n0 in range(0, SHARD_HID, 512):
                    n1 = min(n0 + 512, SHARD_HID)
                    nc.tensor.matmul(
                        hps[:, n0:n1],
                        lhsT=ones_row[:1, :],
                        rhs=b1_sb[:1, n0:n1],
                        start=False,
                        stop=True,
                    )
                h_sb = mpool.tile([P, SHARD_HID], f32, tag="h_sb")
                nc.scalar.activation(
                    out=h_sb[:], in_=hps[:], func=mybir.ActivationFunctionType.Relu
                )

                # hT[p, k2*P + t] = h[t, k2*P + p]
                hT = mpool.tile([P, n_hb * P], f32, tag="hT")
                for k2 in range(n_hb):
                    bs = min(P, SHARD_HID - k2 * P)
                    tp2 = psA.tile([P, P], f32, space="PSUM", tag="tp")
                    nc.tensor.transpose(
                        out=tp2[:bs, :],
                        in_=h_sb[:, k2 * P : k2 * P + bs],
                        identity=identity[:],
                    )
                    nc.vector.tensor_copy(
                        out=hT[:bs, k2 * P : (k2 + 1) * P], in_=tp2[:bs, :]
                    )

                # L2 partial: mlp_part = h_c @ w2_c, computed in two column halves
                HALF = DIM // 2  # 1600 -> 4 PSUM banks
                for hh in range(2):
                    c0 = hh * HALF
                    ops = psO.tile([P, HALF], f32, space="PSUM", tag="ops")
                    for k2 in range(n_hb):
                        bs = min(P, SHARD_HID - k2 * P)
                        w2k = wpool.tile([P, HALF], f32, tag="w2k")
                        nc.sync.dma_start(
                            out=w2k[:bs, :],
                            in_=w2s[k2 * P : k2 * P + bs, c0 : c0 + HALF],
                        )
                        for n0 in range(0, HALF, 512):
                            n1 = min(n0 + 512, HALF)
                            nc.tensor.matmul(
                                ops[:, n0:n1],
                                lhsT=hT[:bs, k2 * P : (k2 + 1) * P],
                                rhs=w2k[:bs, n0:n1],
                                start=(k2 == 0),
                                stop=(k2 == n_hb - 1),
                            )
                    ocp = mpool.tile([P, HALF], f32, tag="ocp")
                    nc.vector.tensor_copy(out=ocp[:], in_=ops[:])
                    nc.sync.dma_start(
                        out=mlp_part[j * P : (j + 1) * P, c0 : c0 + HALF], in_=ocp[:]
                    )

            # ---------------- main gather: 2048 rows/core --------------------
            for t in range(N_T_CHUNKS):
                g = gpool.tile([P, DIM], f32, tag="g")
                nc.gpsimd.indirect_dma_start(
                    out=g[:],
                    out_offset=None,
                    in_=table[:],
                    in_offset=bass.IndirectOffsetOnAxis(
                        ap=idx_sb[:, t : t + 1], axis=0
                    ),
                )
                nc.sync.dma_start(out=out_main[t * P : (t + 1) * P, :], in_=g[:])

    if not nc.is_finalized():
        nc.finalize()
    return nc


def _prepare(inputs):
    """Host-side sharding. Returns (n_mlp_chunks, in_maps, host_ctx)."""
    ids = np.asarray(inputs["input_ids"])
    table = np.ascontiguousarray(np.asarray(inputs["emb_table"], dtype=np.float32))
    w1 = np.asarray(inputs["w1"], dtype=np.float32)
    b1 = np.asarray(inputs["b1"], dtype=np.float32)
    w2 = np.asarray(inputs["w2"], dtype=np.float32)
    b2 = np.asarray(inputs["b2"], dtype=np.float32)

    B, S = ids.shape
    ids_flat = ids.reshape(-1).astype(np.int64)
    N = ids_flat.size
    assert N == N_CORES * T_PER_CORE, (N, ids.shape)

    mask = ids_flat >= NEW_START
    masked_pos = np.nonzero(mask)[0]
    K = int(masked_pos.size)
    n_mlp_chunks = max(1, cdiv(K, P))
    cap = n_mlp_chunks * P
    mids = np.zeros(cap, dtype=np.int64)
    mids[:K] = ids_flat[masked_pos]
    # [P, n_chunks]: column j holds tokens j*128..j*128+127 (token p on partition p)
    mlp_ids_t = np.ascontiguousarray(mids.reshape(n_mlp_chunks, P).T.astype(np.int32))

    in_maps = []
    for c in range(N_CORES):
        idc = ids_flat[c * T_PER_CORE : (c + 1) * T_PER_CORE]
        ids_t = np.ascontiguousarray(idc.reshape(N_T_CHUNKS, P).T.astype(np.int32))
        in_maps.append(
            {
                "ids_t": ids_t,
                "mlp_ids": mlp_ids_t,
                "table": table,
                "w1s": np.ascontiguousarray(
                    w1[:, c * SHARD_HID : (c + 1) * SHARD_HID]
                ),
                "b1s": np.ascontiguousarray(
                    b1[c * SHARD_HID : (c + 1) * SHARD_HID]
                ).reshape(1, SHARD_HID),
                "w2s": np.ascontiguousarray(
                    w2[c * SHARD_HID : (c + 1) * SHARD_HID, :]
                ),
            }
        )
    host_ctx = dict(B=B, S=S, masked_pos=masked_pos, K=K, b2=b2)
    return n_mlp_chunks, in_maps, host_ctx


def _finish(results, host_ctx):
    out = np.concatenate(
        [results[c]["out_main"] for c in range(N_CORES)], axis=0
    )
    K = host_ctx["K"]
    if K > 0:
        mlp = results[0]["mlp_part"].astype(np.float32).copy()
        for c in range(1, N_CORES):
            mlp += results[c]["mlp_part"]
        mlp += host_ctx["b2"][None, :]
        out[host_ctx["masked_pos"]] = mlp[:K]
    return out.reshape(host_ctx["B"], host_ctx["S"], DIM).astype(np.float32)


def kernel(**inputs) -> np.ndarray:
    n_mlp_chunks, in_maps, host_ctx = _prepare(inputs)
    nc = build_program(n_mlp_chunks)
    res = run_bass_kernel_spmd(nc, in_maps, list(range(N_CORES))).results
    return _finish(res, host_ctx)


def kernel_traced(**inputs):
    """Like kernel() but returns (output, BassKernelResults) with profiling."""
    n_mlp_chunks, in_maps, host_ctx = _prepare(inputs)
    nc = build_program(n_mlp_chunks)
    br = run_bass_kernel_spmd(nc, in_maps, list(range(N_CORES)), trace=True)
    return _finish(br.results, host_ctx), br


# revision 16
# speedup vs baseline: 1.7297x; 1.7297x over previous
"""Trainium2 Bass kernel for CustomEmbeddings (embedding lookup + masked MLP).

Computation (reference):
    emb = emb_table[input_ids]                    # [B, S, D]
    mask = input_ids >= 32000
    h = relu(emb @ w1 + b1); mlp = h @ w2 + b2
    out = where(mask, mlp, emb)

Strategy (8 NeuronCores, SPMD — same program, per-core data):
  - Vocab-parallel table sharding with load-balanced boundaries: the host
    picks 8 contiguous vocab ranges holding ~N/8 tokens each (quantiles of
    the id histogram), ships range c to core c, and routes each token to the
    core owning its row.  Core c gathers rows for its tokens (padded to a
    common static T_cap ~ 2048); the host scatters the gathered rows back to
    token positions while unsharding ("shuffle" layout).  This is the
    vocab-parallel hint, but the all-reduce is replaced by host-side routing,
    so the device moves each 12.8KB row exactly once.
  - The masked-token MLP is tiny (~51 tokens expected, all ids >= 32000 live
    in one 100-row slice of the table which is replicated to every core as a
    small side input).  It is weight-sharded 8-way: core c computes
    h[:, c*800:(c+1)*800] = relu(emb@w1_c + b1_c) and the partial
    mlp_out = h_c @ w2_c.  The 8 partials ([K,3200], ~650KB each) are summed
    on the host during unsharding, + b2, and scattered into masked rows.
"""

import sys

if "/opt/trn_rl_repo" not in sys.path:
    sys.path.insert(0, "/opt/trn_rl_repo")

import numpy as np

from concourse import bacc, bass, mybir
import concourse.tile as tile
from concourse.bass_utils import run_bass_kernel_spmd
from concourse.masks import make_identity

P = 128
VOCAB = 32100
DIM = 3200
HID = 6400
NEW_START = 32000
N_CORES = 8
SHARD_HID = HID // N_CORES          # 800
MLP_TAB_ROWS = P                    # replicated new-token slice, ids-NEW_START < 128
N_K_TILES = DIM // P                # 25


def cdiv(a, b):
    return (a + b - 1) // b


# Testing hook: repeat the main gather loop this many times (same data, same
# outputs) so HW wall-clock scaling can separate device time from dispatch
# overhead.  Always 1 in normal use.
GATHER_REPS = 1


def build_program(n_mlp_chunks: int, n_t_chunks: int, s_rows: int) -> bass.Bass:
    f32 = mybir.dt.float32
    i32 = mybir.dt.int32

    # Bacc (not plain Bass): its finalize() runs the wait-legalization passes
    # (move_matmul_waits_to_ldweights / generate_event_semaphores) that split
    # multi-wait instructions the TRN2 ISA encodings cannot carry.
    nc = bacc.Bacc("TRN2")
    ids_t = nc.declare_dram_parameter("ids_t", [P, n_t_chunks], i32, isOutput=False)
    mlp_ids = nc.declare_dram_parameter(
        "mlp_ids", [P, n_mlp_chunks], i32, isOutput=False
    )
    tshard = nc.declare_dram_parameter("tshard", [s_rows, DIM], f32, isOutput=False)
    mlp_tab = nc.declare_dram_parameter(
        "mlp_tab", [MLP_TAB_ROWS, DIM], f32, isOutput=False
    )
    w1s = nc.declare_dram_parameter("w1s", [DIM, SHARD_HID], f32, isOutput=False)
    b1s = nc.declare_dram_parameter("b1s", [1, SHARD_HID], f32, isOutput=False)
    w2s = nc.declare_dram_parameter("w2s", [SHARD_HID, DIM], f32, isOutput=False)
    out_main = nc.declare_dram_parameter(
        "out_main", [n_t_chunks * P, DIM], f32, isOutput=True
    )
    mlp_part = nc.declare_dram_parameter(
        "mlp_part", [n_mlp_chunks * P, DIM], f32, isOutput=True
    )

    n_hb = cdiv(SHARD_HID, P)  # 7 blocks of h columns (6 full + 32)

    with tile.TileContext(nc) as tc:
        with (
            tc.tile_pool(name="const", bufs=1) as consts,
            tc.tile_pool(name="gpool", bufs=6) as gpool,
            tc.tile_pool(name="mpool", bufs=1) as mpool,
            tc.tile_pool(name="wpool", bufs=2) as wpool,
            tc.tile_pool(name="psA", bufs=2, space="PSUM") as psA,
            tc.tile_pool(name="psH", bufs=1, space="PSUM") as psH,
            tc.tile_pool(name="psO", bufs=1, space="PSUM") as psO,
        ):
            ones_row = consts.tile([1, P], f32)
            nc.gpsimd.memset(ones_row[:], 1.0)
            identity = consts.tile([P, P], f32)
            make_identity(nc, identity[:])
            # Priming transpose: the PE transpose lowers to a pure LW
            # instruction that supports only ONE sync wait.  This op makes PE
            # observe the Pool semaphore (identity/ones memsets), so later
            # transposes only wait on their data input.
            prime = psA.tile([P, P], f32, space="PSUM", tag="tp")
            nc.tensor.transpose(out=prime[:], in_=identity[:], identity=identity[:])

            idx_sb = consts.tile([P, n_t_chunks], i32)
            nc.sync.dma_start(out=idx_sb[:], in_=ids_t[:])
            midx_sb = consts.tile([P, n_mlp_chunks], i32)
            nc.sync.dma_start(out=midx_sb[:], in_=mlp_ids[:])
            b1_sb = consts.tile([1, SHARD_HID], f32)
            nc.sync.dma_start(out=b1_sb[:], in_=b1s[:])

            # ---------------- masked-token MLP (small; overlaps with gather) ----
            for j in range(n_mlp_chunks):
                memb = mpool.tile([P, DIM], f32, tag="memb")
                nc.gpsimd.indirect_dma_start(
                    out=memb[:],
                    out_offset=None,
                    in_=mlp_tab[:],
                    in_offset=bass.IndirectOffsetOnAxis(
                        ap=midx_sb[:, j : j + 1], axis=0
                    ),
                )
                # embT[p, k*P + t] = memb[t, k*P + p]
                embT = mpool.tile([P, DIM], f32, tag="embT")
                for k in range(N_K_TILES):
                    tp = psA.tile([P, P], f32, space="PSUM", tag="tp")
                    nc.tensor.transpose(
                        out=tp[:], in_=memb[:, k * P : (k + 1) * P], identity=identity[:]
                    )
                    nc.vector.tensor_copy(out=embT[:, k * P : (k + 1) * P], in_=tp[:])

                # L1: h = relu(emb @ w1s + b1s), h in [tokens, SHARD_HID]
                hps = psH.tile([P, SHARD_HID], f32, space="PSUM", tag="hps")
                for k in range(N_K_TILES):
                    w1k = wpool.tile([P, SHARD_HID], f32, tag="w1k")
                    nc.sync.dma_start(out=w1k[:], in_=w1s[k * P : (k + 1) * P, :])
                    for n0 in range(0, SHARD_HID, 512):
                        n1 = min(n0 + 512, SHARD_HID)
                        nc.tensor.matmul(
                            hps[:, n0:n1],
                            lhsT=embT[:, k * P : (k + 1) * P],
                            rhs=w1k[:, n0:n1],
                            start=(k == 0),
                            stop=False,
                        )
                # bias add as rank-1 update: ones[tokens] x b1[cols]
                for n0 in range(0, SHARD_HID, 512):
                    n1 = min(n0 + 512, SHARD_HID)
                    nc.tensor.matmul(
                        hps[:, n0:n1],
                        lhsT=ones_row[:1, :],
                        rhs=b1_sb[:1, n0:n1],
                        start=False,
                        stop=True,
                    )
                h_sb = mpool.tile([P, SHARD_HID], f32, tag="h_sb")
                nc.scalar.activation(
                    out=h_sb[:], in_=hps[:], func=mybir.ActivationFunctionType.Relu
                )

                # hT[p, k2*P + t] = h[t, k2*P + p]
                hT = mpool.tile([P, n_hb * P], f32, tag="hT")
                for k2 in range(n_hb):
                    bs = min(P, SHARD_HID - k2 * P)
                    tp2 = psA.tile([P, P], f32, space="PSUM", tag="tp")
                    nc.tensor.transpose(
                        out=tp2[:bs, :],
                        in_=h_sb[:, k2 * P : k2 * P + bs],
                        identity=identity[:],
                    )
                    nc.vector.tensor_copy(
                        out=hT[:bs, k2 * P : (k2 + 1) * P], in_=tp2[:bs, :]
                    )

                # L2 partial: mlp_part = h_c @ w2_c, computed in two column halves
                HALF = DIM // 2  # 1600 -> 4 PSUM banks
                for hh in range(2):
                    c0 = hh * HALF
                    ops = psO.tile([P, HALF], f32, space="PSUM", tag="ops")
                    for k2 in range(n_hb):
                        bs = min(P, SHARD_HID - k2 * P)
                        w2k = wpool.tile([P, HALF], f32, tag="w2k")
                        nc.sync.dma_start(
                            out=w2k[:bs, :],
                            in_=w2s[k2 * P : k2 * P + bs, c0 : c0 + HALF],
                        )
                        for n0 in range(0, HALF, 512):
                            n1 = min(n0 + 512, HALF)
                            nc.tensor.matmul(
                                ops[:, n0:n1],
                                lhsT=hT[:bs, k2 * P : (k2 + 1) * P],
                                rhs=w2k[:bs, n0:n1],
                                start=(k2 == 0),
                                stop=(k2 == n_hb - 1),
                            )
                    ocp = mpool.tile([P, HALF], f32, tag="ocp")
                    nc.vector.tensor_copy(out=ocp[:], in_=ops[:])
                    nc.sync.dma_start(
                        out=mlp_part[j * P : (j + 1) * P, c0 : c0 + HALF], in_=ocp[:]
                    )

            # ---------------- main gather: n_t_chunks*128 rows/core -------------
            for t in [t for _ in range(GATHER_REPS) for t in range(n_t_chunks)]:
                g = gpool.tile([P, DIM], f32, tag="g")
                nc.gpsimd.indirect_dma_start(
                    out=g[:],
                    out_offset=None,
                    in_=tshard[:],
                    in_offset=bass.IndirectOffsetOnAxis(
                        ap=idx_sb[:, t : t + 1], axis=0
                    ),
                )
                nc.sync.dma_start(out=out_main[t * P : (t + 1) * P, :], in_=g[:])

    if not nc.is_finalized():
        nc.finalize()
    return nc


def _wrap(ids, n_chunks):
    """[n_chunks*P] -> [P, n_chunks] with element [p, c] = ids[c*P + p]."""
    return np.ascontiguousarray(ids.reshape(n_chunks, P).T.astype(np.int32))


def _prepare(inputs):
    """Host-side sharding. Returns (n_mlp_chunks, n_t_chunks, in_maps, ctx)."""
    ids = np.asarray(inputs["input_ids"])
    table = np.asarray(inputs["emb_table"], dtype=np.float32)
    w1 = np.asarray(inputs["w1"], dtype=np.float32)
    b1 = np.asarray(inputs["b1"], dtype=np.float32)
    w2 = np.asarray(inputs["w2"], dtype=np.float32)
    b2 = np.asarray(inputs["b2"], dtype=np.float32)

    B, S = ids.shape
    ids_flat = ids.reshape(-1).astype(np.int64)
    N = ids_flat.size

    # --- masked tokens (global; same for every core) ---
    mask = ids_flat >= NEW_START
    masked_pos = np.nonzero(mask)[0]
    K = int(masked_pos.size)
    n_mlp_chunks = max(1, cdiv(K, P))
    mids = np.zeros(n_mlp_chunks * P, dtype=np.int64)
    mids[:K] = ids_flat[masked_pos] - NEW_START
    mlp_ids_t = _wrap(mids, n_mlp_chunks)
    mlp_tab = np.zeros((MLP_TAB_ROWS, DIM), dtype=np.float32)
    mlp_tab[: VOCAB - NEW_START] = table[NEW_START:]

    # --- route tokens to vocab shards, exactly balanced ---
    # Sort tokens by id and deal N/8 to each core; core c's table shard is
    # the contiguous row range its tokens span (ranges may overlap at the
    # boundary id — replicating one boundary row per core is free).
    order = np.argsort(ids_flat, kind="stable")
    per = cdiv(N, N_CORES)
    pos_per_core = [order[c * per : (c + 1) * per] for c in range(N_CORES)]
    t_counts = [int(p.size) for p in pos_per_core]
    T_cap = max(P, cdiv(max(t_counts), P) * P)
    n_t_chunks = T_cap // P
    lo_per_core = [int(ids_flat[p[0]]) if p.size else 0 for p in pos_per_core]
    hi_per_core = [int(ids_flat[p[-1]]) + 1 if p.size else 1 for p in pos_per_core]
    s_rows = cdiv(max(h - l for l, h in zip(lo_per_core, hi_per_core)), 16) * 16

    in_maps = []
    for c in range(N_CORES):
        pos_c = pos_per_core[c]
        lo = lo_per_core[c]
        hi = min(lo + s_rows, VOCAB)
        loc = np.zeros(T_cap, dtype=np.int64)
        loc[: pos_c.size] = ids_flat[pos_c] - lo
        tshard = np.zeros((s_rows, DIM), dtype=np.float32)
        tshard[: hi - lo] = table[lo:hi]
        in_maps.append(
            {
                "ids_t": _wrap(loc, n_t_chunks),
                "mlp_ids": mlp_ids_t,
                "tshard": tshard,
                "mlp_tab": mlp_tab,
                "w1s": np.ascontiguousarray(
                    w1[:, c * SHARD_HID : (c + 1) * SHARD_HID]
                ),
                "b1s": np.ascontiguousarray(
                    b1[c * SHARD_HID : (c + 1) * SHARD_HID]
                ).reshape(1, SHARD_HID),
                "w2s": np.ascontiguousarray(
                    w2[c * SHARD_HID : (c + 1) * SHARD_HID, :]
                ),
            }
        )
    ctx = dict(
        B=B, S=S, N=N, masked_pos=masked_pos, K=K, b2=b2,
        pos_per_core=pos_per_core, t_counts=t_counts,
    )
    return n_mlp_chunks, n_t_chunks, s_rows, in_maps, ctx


def _finish(results, ctx):
    out = np.empty((ctx["N"], DIM), dtype=np.float32)
    for c in range(N_CORES):
        tc = ctx["t_counts"][c]
        out[ctx["pos_per_core"][c]] = results[c]["out_main"][:tc]
    K = ctx["K"]
    if K > 0:
        mlp = results[0]["mlp_part"].astype(np.float32).copy()
        for c in range(1, N_CORES):
            mlp += results[c]["mlp_part"]
        mlp += ctx["b2"][None, :]
        out[ctx["masked_pos"]] = mlp[:K]
    return out.reshape(ctx["B"], ctx["S"], DIM)


def kernel(**inputs) -> np.ndarray:
    n_mlp_chunks, n_t_chunks, s_rows, in_maps, ctx = _prepare(inputs)
    nc = build_program(n_mlp_chunks, n_t_chunks, s_rows)
    res = run_bass_kernel_spmd(nc, in_maps, list(range(N_CORES))).results
    return _finish(res, ctx)
